# revision 26
# baseline (speedup 1.0000x reference)
"""Trainium2 Bass kernel for CosineSimilarityWeightedAverage.

reference:
  input [B=4, C=4096, D=64] f32
  in_n = input / ||input||_row
  cos  = in_n @ in_n.T per batch            [B, C, C]
  attn = softmax(cos / 0.1, axis=-1)
  out  = (attn @ weight) * weight_global * input + (attn @ bias) * bias_global

Sharding: 8 cores = (batch b = core//2) x (query half h = core%2, 2048 rows).
Each core gets all 4096 keys of its batch and computes 2048 output rows.

Per-core kernel, fp8 DoubleRow everywhere on the PE:
  - kn8 = e4m3(k / ||k||); 1/||k|| via int rsqrt bit-trick + Newton (DVE only,
    no ACT table). Queries are rows of the same normalized table; the 1/T=10
    temperature and the q-norm fold into the exp scale.
  - knT8 [64, KT*128 (+512 zeros)]: fp8 PE transposes (elem-step-2 psum dst)
    + compacting strided copies rotated over ACT/DVE/Pool.
  - stage 1 (scoresT[k,q] = knT.T @ knT): fp8e4 DoubleRow with a zero-slot
    rhs: slot0 = 512 query cols of knT8, slot1 = the zero block, so the
    64-deep contraction runs at the 0.5 cyc/row DR rate. lhsT slot1 is the
    next k-tile (finite junk killed by the rhs zeros).
  - exp: split across three engines per EXP_PATTERN. ACT: Exp(scale=10,
    bias=-2) -> e5m2 (the -2 shift keeps exp(~11.3) under the e5m2 max and
    cancels in softmax). DVE/Pool: Schraudolph bits round(57.708*raw+48.227)
    written via saturating u8 convert (negatives clamp to exp~0), bitcast
    e5m2.
  - stage 2 + denominator: true DoubleRow over k-tile pairs: lhsT = [W|b]
    e4m3 pairs / ones (stride-16 AP), rhs = e5m2 exp pair-tiles.
  - finalize: acc,den scaled by 2^-12 into f16 (ACT), PE transposes back to
    [q, d], per-partition reciprocal of denT, out = (otW*wg*x + otB*bg)*rinv.
"""

import numpy as np

B = 4
C = 4096
D = 64
NCORES = 8
CQ = C // 2          # queries per core
KT = C // 128        # 32 k-tiles
NJ = KT // 2         # 16 k-tile pairs
QT = CQ // 128       # 16 q-tiles per core

# exp engine split per k-pair j (A=ACT, D=DVE), one pattern per chunk
import os as _os
EXP_PATTERNS = _os.environ.get(
    "EXP_PATTERNS", "ADADADADADADADAA,ADADADADADADADAA,"
    "ADADADADADADADAA,ADADADADADADAADA").split(",")
SQ_ENGINES = _os.environ.get("SQ_ENGINES", "PPPPPPPP")

LN2 = 0.6931471805599453
EXP_SCALE = 10.0
EXP_BIAS = -2.0
SCH_A = EXP_SCALE * 4.0 / LN2                    # 57.70780
SCH_B = 60.0 - 0.2316 + EXP_BIAS * 4.0 / LN2     # 48.2268
FIN_SCALE = 2.0 ** -12

_CACHE = {}


def _build(h):
    import concourse.bass as bass
    import concourse.bacc as bacc
    import concourse.mybir as mybir
    import concourse.tile as tile
    from concourse.masks import make_identity

    f32 = mybir.dt.float32
    f16 = mybir.dt.float16
    f8e4 = mybir.dt.float8e4
    f8e5 = mybir.dt.float8e5
    i32 = mybir.dt.int32
    u8 = mybir.dt.uint8
    u32 = mybir.dt.uint32
    AF = mybir.ActivationFunctionType
    ALU = mybir.AluOpType
    DR = mybir.MatmulPerfMode.DoubleRow

    qt0 = QT * h  # first k-tile of this core's query half

    nc = bacc.Bacc(None, target_bir_lowering=False)
    xk = nc.dram_tensor("xk", [C, D], f32, kind="ExternalInput")
    wcat = nc.dram_tensor("wcat", [C, 2 * D], f32, kind="ExternalInput")
    wg = nc.dram_tensor("wg", [CQ, D], f32, kind="ExternalInput")
    bg = nc.dram_tensor("bg", [CQ, D], f32, kind="ExternalInput")
    out = nc.dram_tensor("out", [CQ, D], f32, kind="ExternalOutput")

    # chunk order: this core's query-half k-tiles first
    corder = [2, 3, 0, 1] if h == 1 else [0, 1, 2, 3]
    torder = [t for c in corder for t in range(8 * c, 8 * c + 8)]
    jorder = [j for c in corder for j in range(4 * c, 4 * c + 4)]

    with tile.TileContext(nc) as tc:
        with (
            tc.tile_pool(name="singles", bufs=1) as singles,
            tc.tile_pool(name="sb", bufs=2) as sb,
            tc.tile_pool(name="exp", bufs=6) as expp,
            tc.tile_pool(name="fin", bufs=2) as fin,
            tc.tile_pool(name="stageA", bufs=1, space="PSUM") as stageA,
            tc.tile_pool(name="stageD", bufs=2, space="PSUM") as stageD,
            tc.tile_pool(name="acc", bufs=1, space="PSUM") as accp,
            tc.tile_pool(name="den", bufs=1, space="PSUM") as denp,
        ):
            # ---------------- tiles ----------------
            xk_r = xk.rearrange("(t p) d -> p t d", p=128)
            wc_r = wcat.rearrange("(t p) m -> p t m", p=128)
            out_r = out.rearrange("(t p) d -> p t d", p=128)
            kbig = singles.tile([128, KT, D], f32)
            wcs = singles.tile([128, KT, 2 * D], f32)
            wgs = singles.tile([128, QT, D], f32)
            bgs = singles.tile([128, QT, D], f32)
            identity = singles.tile([128, 128], f32)
            id4 = singles.tile([128, 128], f8e4)
            id16 = singles.tile([128, 128], f16)
            ones8 = singles.tile([128, 2, 16], f8e5)
            ebias = singles.tile([128, 1], f32)
            knT8 = singles.tile([64, 9 * 512], f8e4)
            knT3 = knT8.rearrange("p (a b) -> p a b", b=512)
            ksq = singles.tile([128, KT], f32)
            rsq = singles.tile([128, KT], f32)
            rtmp = singles.tile([128, KT], f32)
            rtmp2 = singles.tile([128, KT], f32)
            kn8 = singles.tile([128, KT, D], f8e4)
            wb8 = singles.tile([128, NJ, 2, 128], f8e4)
            wb8f = wb8.rearrange("p a t m -> p (a t m)")
            wcsf = wcs.rearrange("p t m -> p (t m)")
            wbq16 = singles.tile([128, QT, 2, D], f16)
            out_nat = singles.tile([128, QT, D], f32)

            # ---------------- DMA: first 8 k-tiles, then wcs c0 ----------
            def dma_kbig(t0, n):
                nc.sync.dma_start(out=kbig[:, t0 : t0 + n, :],
                                  in_=xk_r[:, t0 : t0 + n, :])

            dma_kbig(0, 4)
            dma_kbig(4, 4)
            dma_kbig(8, 8)
            dma_kbig(16, 8)
            dma_kbig(24, 8)
            for c in range(4):
                nc.sync.dma_start(out=wcs[:, 8 * c : 8 * (c + 1), :],
                                  in_=wc_r[:, 8 * c : 8 * (c + 1), :])
            nc.sync.dma_start(out=wgs, in_=wg.rearrange("(t p) d -> p t d", p=128))
            nc.sync.dma_start(out=bgs, in_=bg.rearrange("(t p) d -> p t d", p=128))

            # ---------------- constants ----------------
            make_identity(nc, identity)
            nc.gpsimd.tensor_copy(out=id4, in_=identity)
            nc.gpsimd.tensor_copy(out=id16, in_=identity)
            nc.vector.memset(ones8, 1.0)
            nc.vector.memset(ebias, EXP_BIAS)
            nc.vector.memset(knT8[:, 8 * 512 :].bitcast(u32), 0)

            # ---------------- per-4-tile prep: norms, kn8, transpose -------
            def prep_group(g, sq_engine):
                cs = slice(4 * g, 4 * (g + 1))
                ktmp = sb.tile([128, 4, D], f32, tag="ktmp", name=f"ktmp{g}")
                if sq_engine == "A":
                    nc.scalar.activation(out=ktmp, in_=kbig[:, cs, :],
                                         func=AF.Square)
                else:
                    nc.gpsimd.tensor_mul(ktmp, kbig[:, cs, :], kbig[:, cs, :])
                nc.vector.reduce_sum(out=ksq[:, cs], in_=ktmp,
                                     axis=mybir.AxisListType.X)
                ve = nc.vector
                ve.tensor_scalar(
                    out=rtmp[:, cs].bitcast(i32), in0=ksq[:, cs].bitcast(i32),
                    scalar1=1, scalar2=None, op0=ALU.arith_shift_right)
                ve.tensor_scalar(
                    out=rsq[:, cs].bitcast(i32), in0=rtmp[:, cs].bitcast(i32),
                    scalar1=-1, scalar2=0x5F3759DF, op0=ALU.mult, op1=ALU.add)
                ve.tensor_mul(rtmp[:, cs], rsq[:, cs], rsq[:, cs])
                ve.tensor_mul(rtmp2[:, cs], rtmp[:, cs], ksq[:, cs])
                ve.tensor_scalar(
                    out=rtmp2[:, cs], in0=rtmp2[:, cs],
                    scalar1=-0.5, scalar2=1.5, op0=ALU.mult, op1=ALU.add)
                ve.tensor_mul(rsq[:, cs], rsq[:, cs], rtmp2[:, cs])
                nc.vector.tensor_mul(
                    kn8[:, cs, :], kbig[:, cs, :],
                    rsq[:, cs].unsqueeze(2).broadcast_to([128, 4, D]))
                trp = stageA.tile([64, 4, 128, 2], f8e4, tag="stageA",
                                  name=f"trp{g}")
                for s in range(4):
                    nc.tensor.transpose(trp[:, s, :, 0], kn8[:, 4 * g + s, :], id4)
                dst = knT8[:, 4 * g * 128 : (4 * g + 4) * 128]
                if g % 4 != 3:
                    nc.scalar.copy(out=dst, in_=trp[:, :, :, 0])
                else:
                    nc.vector.tensor_copy(out=dst, in_=trp[:, :, :, 0])

            def wb_cast(c):
                cs = slice(512 * c, 512 * (c + 1))
                nc.gpsimd.tensor_copy(out=wb8f[:, cs], in_=wcsf[:, cs])

            # groups 0-1 + first weight cast up front
            prep_group(0, SQ_ENGINES[0])
            prep_group(1, SQ_ENGINES[1])
            wb_cast(0)
            for g in range(2, 8):
                prep_group(g, SQ_ENGINES[g])
                if g - 1 <= 7:
                    wb_cast(g - 1)
            wb_cast(7)
            nc.gpsimd.tensor_mul(wbq16[:, :, 0, :], wgs, kbig[:, qt0 : qt0 + QT, :])
            nc.gpsimd.tensor_copy(out=wbq16[:, :, 1, :], in_=bgs)

            # ---------------- finalize ----------------
            def fin_phase2(qc, half=None):
                acc16, den16 = fins[qc]
                key = f"{qc}" if half is None else f"{qc}_{half}"
                ot = stageD.tile([128, 512], f32, tag="stageD",
                                 name=f"ot{key}")
                ot16 = ot.bitcast(f16)
                srange = (0, 1, 2, 3) if half is None else (2 * half, 2 * half + 1)
                for s in srange:
                    nc.tensor.transpose(
                        ot16[:, 128 * s : 128 * (s + 1)],
                        acc16[:, 128 * s : 128 * (s + 1)], id16)
                    nc.tensor.transpose(
                        ot16[:, 512 + 2 * s : 512 + 2 * s + 1],
                        den16[:, 128 * s : 128 * (s + 1)], id16[0:1, 0:1])
                nq = len(srange)
                s0_, q0_ = srange[0], 4 * qc + srange[0]
                rinvT = fin.tile([128, nq], f32, tag="rinvT", name=f"ri{key}")
                nc.vector.reciprocal(
                    out=rinvT,
                    in_=ot16[:, 512 + 2 * s0_ : 512 + 2 * s0_ + 2 * nq : 2])
                prod = fin.tile([128, nq, 128], f16, tag="prod", name=f"pr{key}")
                nc.vector.tensor_mul(
                    prod,
                    ot16[:, 128 * s0_ : 128 * (s0_ + nq)].rearrange(
                        "p (s m) -> p s m", s=nq),
                    wbq16[:, q0_ : q0_ + nq, :, :].rearrange(
                        "p a t m -> p a (t m)"))
                sumh = fin.tile([128, nq, D], f16, tag="sumh", name=f"sh{key}")
                nc.vector.tensor_add(sumh, prod[:, :, 0:D], prod[:, :, D:])
                nc.vector.tensor_mul(
                    out_nat[:, q0_ : q0_ + nq, :], sumh,
                    rinvT.unsqueeze(2).broadcast_to([128, nq, D]))
                nc.sync.dma_start(out=out_r[:, q0_ : q0_ + nq, :],
                                  in_=out_nat[:, q0_ : q0_ + nq, :])

            # ---------------- main loop ----------------
            fins = {}
            for qc in range(4):
                s0 = 4 * h + qc
                rhs = knT3[:, s0 : 9 : 8 - s0, :]
                acc_ps = accp.tile([128, 512], f32, tag="acc", name=f"acc{qc}")
                den_ps = denp.tile([1, 512], f32, tag="den", name=f"den{qc}")

                # schedule: group adjacent A-pairs in the pattern into quads
                pat = EXP_PATTERNS[qc]
                sched = []
                i = 0
                while i < NJ:
                    if pat[i] == "A" and i + 1 < NJ and pat[i + 1] == "A":
                        sched.append(("Q", (jorder[i], jorder[i + 1])))
                        i += 2
                    elif pat[i] == "A":
                        sched.append(("A", (jorder[i],)))
                        i += 1
                    else:
                        sched.append(("D", (jorder[i],)))
                        i += 1

                def lhsTs(j):
                    return (
                        knT8[:, 256 * j : 256 * j + 256].rearrange(
                            "p (t m) -> p t m", t=2),
                        knT8[:, 256 * j + 128 : 256 * j + 384].rearrange(
                            "p (t m) -> p t m", t=2))

                done = 0
                pending = []
                fin_emitted = qc == 0
                for kind, js in sched + [(None, ())]:
                    # drain stage-2/den for every e-pair already produced
                    for pj, e in pending:
                        done += 1
                        nc.tensor.matmul(
                            den_ps, lhsT=ones8[:, :, 0:1], rhs=e,
                            start=(done == 1), stop=(done == NJ),
                            perf_mode=DR, skip_group_check=True)
                        nc.tensor.matmul(
                            acc_ps, lhsT=wb8[:, pj], rhs=e,
                            start=(done == 1), stop=(done == NJ),
                            perf_mode=DR, skip_group_check=True)
                    pending = []
                    if not fin_emitted and done >= 7:
                        fin_phase2(qc - 1)
                        fin_emitted = True
                    if kind is None:
                        break
                    if kind == "Q":
                        j0, j1 = js
                        st = stageA.tile([128, 4, 512], f32, tag="stageA",
                                         name=f"st{qc}_{j0}q")
                        eq = expp.tile([128, 4, 512], f8e5, tag="exp",
                                       name=f"e{qc}_{j0}q")
                        for s, (jj, par) in enumerate(
                                ((j0, 0), (j0, 1), (j1, 0), (j1, 1))):
                            nc.tensor.matmul(
                                st[:, s, :], lhsT=lhsTs(jj)[par], rhs=rhs,
                                start=True, stop=True, perf_mode=DR)
                        nc.scalar.activation(out=eq, in_=st, func=AF.Exp,
                                             bias=ebias[:, 0:1],
                                             scale=EXP_SCALE)
                        pending = [(j0, eq[:, 0:2, :]), (j1, eq[:, 2:4, :])]
                    elif kind == "A":
                        (j,) = js
                        st = stageA.tile([128, 4, 512], f32, tag="stageA",
                                         name=f"st{qc}_{j}p")
                        e = expp.tile([128, 2, 512], f8e5, tag="exp",
                                      name=f"e{qc}_{j}")
                        l0, l1 = lhsTs(j)
                        nc.tensor.matmul(st[:, 0, :], lhsT=l0, rhs=rhs,
                                         start=True, stop=True, perf_mode=DR)
                        nc.tensor.matmul(st[:, 1, :], lhsT=l1, rhs=rhs,
                                         start=True, stop=True, perf_mode=DR)
                        nc.scalar.activation(out=e, in_=st[:, 0:2, :],
                                             func=AF.Exp, bias=ebias[:, 0:1],
                                             scale=EXP_SCALE)
                        pending = [(j, e)]
                    else:
                        (j,) = js
                        e = expp.tile([128, 2, 512], f8e5, tag="exp",
                                      name=f"e{qc}_{j}")
                        l0, l1 = lhsTs(j)
                        for par, lh in ((0, l0), (1, l1)):
                            stx = stageD.tile([128, 512], f32, tag="stageD",
                                              name=f"st{qc}_{j}_{par}")
                            nc.tensor.matmul(stx, lhsT=lh, rhs=rhs,
                                             start=True, stop=True,
                                             perf_mode=DR)
                            nc.vector.tensor_scalar(
                                out=e[:, par, :].bitcast(u8), in0=stx,
                                scalar1=SCH_A, scalar2=SCH_B,
                                op0=ALU.mult, op1=ALU.add)
                        pending = [(j, e)]

                den16 = fin.tile([1, 512], f16, tag="den16", name=f"d16_{qc}")
                nc.scalar.activation(out=den16, in_=den_ps, func=AF.Copy,
                                     scale=FIN_SCALE)
                acc16 = fin.tile([128, 512], f16, tag="acc16", name=f"a16_{qc}")
                nc.scalar.activation(out=acc16, in_=acc_ps, func=AF.Copy,
                                     scale=FIN_SCALE)
                fins[qc] = (acc16, den16)
            fin_phase2(3, half=0)
            fin_phase2(3, half=1)

    nc.compile()
    return nc


def _get_nc(h):
    key = f"nc{h}"
    if key not in _CACHE:
        _CACHE[key] = _build(h)
    return _CACHE[key]


def _make_in_maps(input, weight, bias, weight_global, bias_global):
    input = np.ascontiguousarray(np.asarray(input, dtype=np.float32))
    ones = lambda: np.ones((C, D), np.float32)
    weight = ones() if weight is None else np.asarray(weight, np.float32)
    bias = np.zeros((C, D), np.float32) if bias is None else np.asarray(bias, np.float32)
    weight_global = ones() if weight_global is None else np.asarray(weight_global, np.float32)
    bias_global = ones() if bias_global is None else np.asarray(bias_global, np.float32)
    wcat = np.ascontiguousarray(np.concatenate([weight, bias], axis=1))
    in_maps = []
    for core in range(NCORES):
        b, h = divmod(core, 2)
        sl = slice(h * CQ, (h + 1) * CQ)
        in_maps.append({
            "xk": np.ascontiguousarray(input[b]),
            "wcat": wcat,
            "wg": np.ascontiguousarray(weight_global[sl]),
            "bg": np.ascontiguousarray(bias_global[sl]),
        })
    return in_maps


def _run(in_maps, **kw):
    from concourse.bass_utils import run_bass_kernel_spmd
    # h differs per core but all cores run one SPMD module; build for h via
    # per-core modules is not supported by run_bass_kernel_spmd -- so we run
    # the two halves as the same module parameterized by a dram flag? No:
    # instead we exploit that run_bass_kernel_spmd takes ONE nc. We fold h
    # into the inputs by rolling xk so every core behaves like h=0.
    nc = _get_nc(0)
    return run_bass_kernel_spmd(nc, in_maps, core_ids=list(range(NCORES)), **kw)


def kernel(input, weight=None, bias=None, weight_global=None, bias_global=None,
           **_ignored):
    in_maps = _make_in_maps(input, weight, bias, weight_global, bias_global)
    # roll keys so each core's query half sits at k-tiles 0..15
    for core in range(NCORES):
        b, h = divmod(core, 2)
        if h == 1:
            m = in_maps[core]
            m["xk"] = np.ascontiguousarray(np.roll(m["xk"], -CQ, axis=0))
            m["wcat"] = np.ascontiguousarray(np.roll(m["wcat"], -CQ, axis=0))
    res = _run(in_maps)
    out = np.empty((B, C, D), np.float32)
    for core in range(NCORES):
        b, h = divmod(core, 2)
        out[b, h * CQ : (h + 1) * CQ] = res.results[core]["out"]
    return out


# revision 27
# speedup vs baseline: 1.4028x; 1.4028x over previous
"""Trainium2 Bass kernel for CosineSimilarityWeightedAverage.

reference:
  input [B=4, C=4096, D=64] f32
  in_n = input / ||input||_row
  cos  = in_n @ in_n.T per batch            [B, C, C]
  attn = softmax(cos / 0.1, axis=-1)
  out  = (attn @ weight) * weight_global * input + (attn @ bias) * bias_global

Sharding: 8 cores = (batch b = core//2) x (query half h = core%2, 2048 rows).
Each core gets all 4096 keys of its batch and computes 2048 output rows.

Per-core kernel, fp8 DoubleRow everywhere on the PE:
  - kn8 = e4m3(k / ||k||); 1/||k|| via int rsqrt bit-trick + Newton (DVE only,
    no ACT table). Queries are rows of the same normalized table; the 1/T=10
    temperature and the q-norm fold into the exp scale.
  - knT8 [64, KT*128 (+512 zeros)]: fp8 PE transposes (elem-step-2 psum dst)
    + compacting strided copies rotated over ACT/DVE/Pool.
  - stage 1 (scoresT[k,q] = knT.T @ knT): fp8e4 DoubleRow with a zero-slot
    rhs: slot0 = 512 query cols of knT8, slot1 = the zero block, so the
    64-deep contraction runs at the 0.5 cyc/row DR rate. lhsT slot1 is the
    next k-tile (finite junk killed by the rhs zeros).
  - exp: split across three engines per EXP_PATTERN. ACT: Exp(scale=10,
    bias=-2) -> e5m2 (the -2 shift keeps exp(~11.3) under the e5m2 max and
    cancels in softmax). DVE/Pool: Schraudolph bits round(57.708*raw+48.227)
    written via saturating u8 convert (negatives clamp to exp~0), bitcast
    e5m2.
  - stage 2 + denominator: true DoubleRow over k-tile pairs: lhsT = [W|b]
    e4m3 pairs / ones (stride-16 AP), rhs = e5m2 exp pair-tiles.
  - finalize: acc,den scaled by 2^-12 into f16 (ACT), PE transposes back to
    [q, d], per-partition reciprocal of denT, out = (otW*wg*x + otB*bg)*rinv.
"""

import numpy as np

B = 4
C = 4096
D = 64
NCORES = 8
CQ = C // 2          # queries per core
KT = C // 128        # 32 k-tiles
NJ = KT // 2         # 16 k-tile pairs
QT = CQ // 128       # 16 q-tiles per core

# exp engine split per k-pair j (A=ACT, D=DVE), one pattern per chunk
import os as _os
EXP_PATTERNS = _os.environ.get(
    "EXP_PATTERNS", "ADADADADADADADAA,ADADADADADADADAA,"
    "ADADADADADADADAA,ADADADADADADAADA").split(",")
SQ_ENGINES = _os.environ.get("SQ_ENGINES", "PPPPPPPP")

LN2 = 0.6931471805599453
EXP_SCALE = 10.0
EXP_BIAS = -2.0
SCH_A = EXP_SCALE * 4.0 / LN2                    # 57.70780
SCH_B = 60.0 - 0.2316 + EXP_BIAS * 4.0 / LN2     # 48.2268
FIN_SCALE = 2.0 ** -12

_CACHE = {}


def _build(h):
    import concourse.bass as bass
    import concourse.bacc as bacc
    import concourse.mybir as mybir
    import concourse.tile as tile
    from concourse.masks import make_identity

    f32 = mybir.dt.float32
    f16 = mybir.dt.float16
    f8e4 = mybir.dt.float8e4
    f8e5 = mybir.dt.float8e5
    i32 = mybir.dt.int32
    u8 = mybir.dt.uint8
    u32 = mybir.dt.uint32
    AF = mybir.ActivationFunctionType
    ALU = mybir.AluOpType
    DR = mybir.MatmulPerfMode.DoubleRow

    qt0 = QT * h  # first k-tile of this core's query half

    nc = bacc.Bacc(None, target_bir_lowering=False)
    xk = nc.dram_tensor("xk", [C, D], f32, kind="ExternalInput")
    wcat = nc.dram_tensor("wcat", [C, 2 * D], f32, kind="ExternalInput")
    wg = nc.dram_tensor("wg", [CQ, D], f32, kind="ExternalInput")
    bg = nc.dram_tensor("bg", [CQ, D], f32, kind="ExternalInput")
    out = nc.dram_tensor("out", [CQ, D], f32, kind="ExternalOutput")

    # chunk order: this core's query-half k-tiles first
    corder = [2, 3, 0, 1] if h == 1 else [0, 1, 2, 3]
    torder = [t for c in corder for t in range(8 * c, 8 * c + 8)]
    jorder = [j for c in corder for j in range(4 * c, 4 * c + 4)]

    with tile.TileContext(nc) as tc:
        with (
            tc.tile_pool(name="singles", bufs=1) as singles,
            tc.tile_pool(name="sb", bufs=2) as sb,
            tc.tile_pool(name="exp", bufs=6) as expp,
            tc.tile_pool(name="fin", bufs=2) as fin,
            tc.tile_pool(name="stageA", bufs=2, space="PSUM") as stageA,
            tc.tile_pool(name="stageD", bufs=2, space="PSUM") as stageD,
            tc.tile_pool(name="acc", bufs=1, space="PSUM") as accp,
            tc.tile_pool(name="den", bufs=1, space="PSUM") as denp,
        ):
            # ---------------- tiles ----------------
            xk_r = xk.rearrange("(t p) d -> p t d", p=128)
            wc_r = wcat.rearrange("(t p) m -> p t m", p=128)
            out_r = out.rearrange("(t p) d -> p t d", p=128)
            kbig = singles.tile([128, KT, D], f32)
            wcs = singles.tile([128, KT, 2 * D], f32)
            wgs = singles.tile([128, QT, D], f32)
            bgs = singles.tile([128, QT, D], f32)
            identity = singles.tile([128, 128], f32)
            id4 = singles.tile([128, 128], f8e4)
            id16 = singles.tile([128, 128], f16)
            ones8 = singles.tile([128, 2, 16], f8e5)
            ebias = singles.tile([128, 1], f32)
            knT8 = singles.tile([64, 9 * 512], f8e4)
            knT3 = knT8.rearrange("p (a b) -> p a b", b=512)
            ksq = singles.tile([128, KT], f32)
            rsq = singles.tile([128, KT], f32)
            rtmp = singles.tile([128, KT], f32)
            rtmp2 = singles.tile([128, KT], f32)
            kn8 = singles.tile([128, KT, D], f8e4)
            wb8 = singles.tile([128, NJ, 2, 128], f8e4)
            wb8f = wb8.rearrange("p a t m -> p (a t m)")
            wcsf = wcs.rearrange("p t m -> p (t m)")
            wbq16 = singles.tile([128, QT, 2, D], f16)
            out_nat = singles.tile([128, QT, D], f32)

            # ---------------- DMA: first 8 k-tiles, then wcs c0 ----------
            def dma_kbig(t0, n):
                nc.sync.dma_start(out=kbig[:, t0 : t0 + n, :],
                                  in_=xk_r[:, t0 : t0 + n, :])

            dma_kbig(0, 4)
            dma_kbig(4, 4)
            dma_kbig(8, 8)
            dma_kbig(16, 8)
            dma_kbig(24, 8)
            for c in range(4):
                nc.sync.dma_start(out=wcs[:, 8 * c : 8 * (c + 1), :],
                                  in_=wc_r[:, 8 * c : 8 * (c + 1), :])
            nc.sync.dma_start(out=wgs, in_=wg.rearrange("(t p) d -> p t d", p=128))
            nc.sync.dma_start(out=bgs, in_=bg.rearrange("(t p) d -> p t d", p=128))

            # ---------------- constants ----------------
            make_identity(nc, identity)
            nc.gpsimd.tensor_copy(out=id4, in_=identity)
            nc.gpsimd.tensor_copy(out=id16, in_=identity)
            nc.vector.memset(ones8, 1.0)
            nc.vector.memset(ebias, EXP_BIAS)
            nc.vector.memset(knT8[:, 8 * 512 :].bitcast(u32), 0)

            # ---------------- per-4-tile prep: norms, kn8, transpose -------
            def prep_group(g, sq_engine):
                cs = slice(4 * g, 4 * (g + 1))
                ktmp = sb.tile([128, 4, D], f32, tag="ktmp", name=f"ktmp{g}")
                if sq_engine == "A":
                    nc.scalar.activation(out=ktmp, in_=kbig[:, cs, :],
                                         func=AF.Square)
                else:
                    nc.gpsimd.tensor_mul(ktmp, kbig[:, cs, :], kbig[:, cs, :])
                nc.vector.reduce_sum(out=ksq[:, cs], in_=ktmp,
                                     axis=mybir.AxisListType.X)
                ve = nc.vector
                ve.tensor_scalar(
                    out=rtmp[:, cs].bitcast(i32), in0=ksq[:, cs].bitcast(i32),
                    scalar1=1, scalar2=None, op0=ALU.arith_shift_right)
                ve.tensor_scalar(
                    out=rsq[:, cs].bitcast(i32), in0=rtmp[:, cs].bitcast(i32),
                    scalar1=-1, scalar2=0x5F3759DF, op0=ALU.mult, op1=ALU.add)
                ve.tensor_mul(rtmp[:, cs], rsq[:, cs], rsq[:, cs])
                ve.tensor_mul(rtmp2[:, cs], rtmp[:, cs], ksq[:, cs])
                ve.tensor_scalar(
                    out=rtmp2[:, cs], in0=rtmp2[:, cs],
                    scalar1=-0.5, scalar2=1.5, op0=ALU.mult, op1=ALU.add)
                ve.tensor_mul(rsq[:, cs], rsq[:, cs], rtmp2[:, cs])
                nc.vector.tensor_mul(
                    kn8[:, cs, :], kbig[:, cs, :],
                    rsq[:, cs].unsqueeze(2).broadcast_to([128, 4, D]))
                trp = stageA.tile([64, 4, 128, 2], f8e4, tag="stageA",
                                  name=f"trp{g}")
                for s in range(4):
                    nc.tensor.transpose(trp[:, s, :, 0], kn8[:, 4 * g + s, :], id4)
                dst = knT8[:, 4 * g * 128 : (4 * g + 4) * 128]
                if g % 4 != 3:
                    nc.scalar.copy(out=dst, in_=trp[:, :, :, 0])
                else:
                    nc.vector.tensor_copy(out=dst, in_=trp[:, :, :, 0])

            def wb_cast(c):
                cs = slice(512 * c, 512 * (c + 1))
                nc.gpsimd.tensor_copy(out=wb8f[:, cs], in_=wcsf[:, cs])

            # groups 0-1 + first weight cast up front
            prep_group(0, SQ_ENGINES[0])
            prep_group(1, SQ_ENGINES[1])
            wb_cast(0)
            for g in range(2, 8):
                prep_group(g, SQ_ENGINES[g])
                if g - 1 <= 7:
                    wb_cast(g - 1)
            wb_cast(7)
            nc.gpsimd.tensor_mul(wbq16[:, :, 0, :], wgs, kbig[:, qt0 : qt0 + QT, :])
            nc.gpsimd.tensor_copy(out=wbq16[:, :, 1, :], in_=bgs)

            # ---------------- finalize ----------------
            def fin_phase2(qc, half=None):
                acc16, den16 = fins[qc]
                key = f"{qc}" if half is None else f"{qc}_{half}"
                ot = stageD.tile([128, 512], f32, tag="stageD",
                                 name=f"ot{key}")
                ot16 = ot.bitcast(f16)
                srange = (0, 1, 2, 3) if half is None else (2 * half, 2 * half + 1)
                for s in srange:
                    nc.tensor.transpose(
                        ot16[:, 128 * s : 128 * (s + 1)],
                        acc16[:, 128 * s : 128 * (s + 1)], id16)
                    nc.tensor.transpose(
                        ot16[:, 512 + 2 * s : 512 + 2 * s + 1],
                        den16[:, 128 * s : 128 * (s + 1)], id16[0:1, 0:1])
                nq = len(srange)
                s0_, q0_ = srange[0], 4 * qc + srange[0]
                rinvT = fin.tile([128, nq], f32, tag="rinvT", name=f"ri{key}")
                nc.vector.reciprocal(
                    out=rinvT,
                    in_=ot16[:, 512 + 2 * s0_ : 512 + 2 * s0_ + 2 * nq : 2])
                prod = fin.tile([128, nq, 128], f16, tag="prod", name=f"pr{key}")
                nc.vector.tensor_mul(
                    prod,
                    ot16[:, 128 * s0_ : 128 * (s0_ + nq)].rearrange(
                        "p (s m) -> p s m", s=nq),
                    wbq16[:, q0_ : q0_ + nq, :, :].rearrange(
                        "p a t m -> p a (t m)"))
                sumh = fin.tile([128, nq, D], f16, tag="sumh", name=f"sh{key}")
                nc.vector.tensor_add(sumh, prod[:, :, 0:D], prod[:, :, D:])
                nc.vector.tensor_mul(
                    out_nat[:, q0_ : q0_ + nq, :], sumh,
                    rinvT.unsqueeze(2).broadcast_to([128, nq, D]))
                nc.sync.dma_start(out=out_r[:, q0_ : q0_ + nq, :],
                                  in_=out_nat[:, q0_ : q0_ + nq, :])

            # ---------------- main loop ----------------
            fins = {}
            for qc in range(4):
                s0 = 4 * h + qc
                rhs = knT3[:, s0 : 9 : 8 - s0, :]
                acc_ps = accp.tile([128, 512], f32, tag="acc", name=f"acc{qc}")
                den_ps = denp.tile([1, 512], f32, tag="den", name=f"den{qc}")

                # schedule: group adjacent A-pairs in the pattern into quads
                pat = EXP_PATTERNS[qc]
                sched = []
                i = 0
                while i < NJ:
                    if False and pat[i] == "A" and i + 1 < NJ and pat[i + 1] == "A":
                        sched.append(("Q", (jorder[i], jorder[i + 1])))
                        i += 2
                    elif pat[i] == "A":
                        sched.append(("A", (jorder[i],)))
                        i += 1
                    else:
                        sched.append(("D", (jorder[i],)))
                        i += 1

                def lhsTs(j):
                    return (
                        knT8[:, 256 * j : 256 * j + 256].rearrange(
                            "p (t m) -> p t m", t=2),
                        knT8[:, 256 * j + 128 : 256 * j + 384].rearrange(
                            "p (t m) -> p t m", t=2))

                done = 0
                pending = []
                fin_emitted = qc == 0
                for kind, js in sched + [(None, ())]:
                    # drain stage-2/den for every e-pair already produced
                    for pj, e in pending:
                        done += 1
                        nc.tensor.matmul(
                            den_ps, lhsT=ones8[:, :, 0:1], rhs=e,
                            start=(done == 1), stop=(done == NJ),
                            perf_mode=DR, skip_group_check=True)
                        nc.tensor.matmul(
                            acc_ps, lhsT=wb8[:, pj], rhs=e,
                            start=(done == 1), stop=(done == NJ),
                            perf_mode=DR, skip_group_check=True)
                    pending = []
                    if not fin_emitted and done >= 7:
                        fin_phase2(qc - 1)
                        fin_emitted = True
                    if kind is None:
                        break
                    if kind == "Q":
                        j0, j1 = js
                        st = stageA.tile([128, 4, 512], f32, tag="stageA",
                                         name=f"st{qc}_{j0}q")
                        eq = expp.tile([128, 4, 512], f8e5, tag="exp",
                                       name=f"e{qc}_{j0}q")
                        for s, (jj, par) in enumerate(
                                ((j0, 0), (j0, 1), (j1, 0), (j1, 1))):
                            nc.tensor.matmul(
                                st[:, s, :], lhsT=lhsTs(jj)[par], rhs=rhs,
                                start=True, stop=True, perf_mode=DR)
                        nc.scalar.activation(out=eq, in_=st, func=AF.Exp,
                                             bias=ebias[:, 0:1],
                                             scale=EXP_SCALE)
                        pending = [(j0, eq[:, 0:2, :]), (j1, eq[:, 2:4, :])]
                    elif kind == "A":
                        (j,) = js
                        st = stageA.tile([128, 2, 512], f32, tag="stageA",
                                         name=f"st{qc}_{j}p")
                        e = expp.tile([128, 2, 512], f8e5, tag="exp",
                                      name=f"e{qc}_{j}")
                        l0, l1 = lhsTs(j)
                        nc.tensor.matmul(st[:, 0, :], lhsT=l0, rhs=rhs,
                                         start=True, stop=True, perf_mode=DR)
                        nc.tensor.matmul(st[:, 1, :], lhsT=l1, rhs=rhs,
                                         start=True, stop=True, perf_mode=DR)
                        nc.scalar.activation(out=e, in_=st, func=AF.Exp,
                                             bias=ebias[:, 0:1],
                                             scale=EXP_SCALE)
                        pending = [(j, e)]
                    else:
                        (j,) = js
                        e = expp.tile([128, 2, 512], f8e5, tag="exp",
                                      name=f"e{qc}_{j}")
                        l0, l1 = lhsTs(j)
                        for par, lh in ((0, l0), (1, l1)):
                            stx = stageD.tile([128, 512], f32, tag="stageD",
                                              name=f"st{qc}_{j}_{par}")
                            nc.tensor.matmul(stx, lhsT=lh, rhs=rhs,
                                             start=True, stop=True,
                                             perf_mode=DR)
                            nc.vector.tensor_scalar(
                                out=e[:, par, :].bitcast(u8), in0=stx,
                                scalar1=SCH_A, scalar2=SCH_B,
                                op0=ALU.mult, op1=ALU.add)
                        pending = [(j, e)]

                den16 = fin.tile([1, 512], f16, tag="den16", name=f"d16_{qc}")
                nc.scalar.activation(out=den16, in_=den_ps, func=AF.Copy,
                                     scale=FIN_SCALE)
                acc16 = fin.tile([128, 512], f16, tag="acc16", name=f"a16_{qc}")
                nc.scalar.activation(out=acc16, in_=acc_ps, func=AF.Copy,
                                     scale=FIN_SCALE)
                fins[qc] = (acc16, den16)
            fin_phase2(3, half=0)
            fin_phase2(3, half=1)

    nc.compile()
    return nc


def _get_nc(h):
    key = f"nc{h}"
    if key not in _CACHE:
        _CACHE[key] = _build(h)
    return _CACHE[key]


def _make_in_maps(input, weight, bias, weight_global, bias_global):
    input = np.ascontiguousarray(np.asarray(input, dtype=np.float32))
    ones = lambda: np.ones((C, D), np.float32)
    weight = ones() if weight is None else np.asarray(weight, np.float32)
    bias = np.zeros((C, D), np.float32) if bias is None else np.asarray(bias, np.float32)
    weight_global = ones() if weight_global is None else np.asarray(weight_global, np.float32)
    bias_global = ones() if bias_global is None else np.asarray(bias_global, np.float32)
    wcat = np.ascontiguousarray(np.concatenate([weight, bias], axis=1))
    in_maps = []
    for core in range(NCORES):
        b, h = divmod(core, 2)
        sl = slice(h * CQ, (h + 1) * CQ)
        in_maps.append({
            "xk": np.ascontiguousarray(input[b]),
            "wcat": wcat,
            "wg": np.ascontiguousarray(weight_global[sl]),
            "bg": np.ascontiguousarray(bias_global[sl]),
        })
    return in_maps


def _run(in_maps, **kw):
    from concourse.bass_utils import run_bass_kernel_spmd
    # h differs per core but all cores run one SPMD module; build for h via
    # per-core modules is not supported by run_bass_kernel_spmd -- so we run
    # the two halves as the same module parameterized by a dram flag? No:
    # instead we exploit that run_bass_kernel_spmd takes ONE nc. We fold h
    # into the inputs by rolling xk so every core behaves like h=0.
    nc = _get_nc(0)
    return run_bass_kernel_spmd(nc, in_maps, core_ids=list(range(NCORES)), **kw)


def kernel(input, weight=None, bias=None, weight_global=None, bias_global=None,
           **_ignored):
    in_maps = _make_in_maps(input, weight, bias, weight_global, bias_global)
    # roll keys so each core's query half sits at k-tiles 0..15
    for core in range(NCORES):
        b, h = divmod(core, 2)
        if h == 1:
            m = in_maps[core]
            m["xk"] = np.ascontiguousarray(np.roll(m["xk"], -CQ, axis=0))
            m["wcat"] = np.ascontiguousarray(np.roll(m["wcat"], -CQ, axis=0))
    res = _run(in_maps)
    out = np.empty((B, C, D), np.float32)
    for core in range(NCORES):
        b, h = divmod(core, 2)
        out[b, h * CQ : (h + 1) * CQ] = res.results[core]["out"]
    return out


# revision 28
# speedup vs baseline: 1.4162x; 1.0096x over previous
"""Trainium2 Bass kernel for CosineSimilarityWeightedAverage.

reference:
  input [B=4, C=4096, D=64] f32
  in_n = input / ||input||_row
  cos  = in_n @ in_n.T per batch            [B, C, C]
  attn = softmax(cos / 0.1, axis=-1)
  out  = (attn @ weight) * weight_global * input + (attn @ bias) * bias_global

Sharding: 8 cores = (batch b = core//2) x (query half h = core%2, 2048 rows).
Each core gets all 4096 keys of its batch and computes 2048 output rows.

Per-core kernel, fp8 DoubleRow everywhere on the PE:
  - kn8 = e4m3(k / ||k||); 1/||k|| via int rsqrt bit-trick + Newton (DVE only,
    no ACT table). Queries are rows of the same normalized table; the 1/T=10
    temperature and the q-norm fold into the exp scale.
  - knT8 [64, KT*128 (+512 zeros)]: fp8 PE transposes (elem-step-2 psum dst)
    + compacting strided copies rotated over ACT/DVE/Pool.
  - stage 1 (scoresT[k,q] = knT.T @ knT): fp8e4 DoubleRow with a zero-slot
    rhs: slot0 = 512 query cols of knT8, slot1 = the zero block, so the
    64-deep contraction runs at the 0.5 cyc/row DR rate. lhsT slot1 is the
    next k-tile (finite junk killed by the rhs zeros).
  - exp: split across three engines per EXP_PATTERN. ACT: Exp(scale=10,
    bias=-2) -> e5m2 (the -2 shift keeps exp(~11.3) under the e5m2 max and
    cancels in softmax). DVE/Pool: Schraudolph bits round(57.708*raw+48.227)
    written via saturating u8 convert (negatives clamp to exp~0), bitcast
    e5m2.
  - stage 2 + denominator: true DoubleRow over k-tile pairs: lhsT = [W|b]
    e4m3 pairs / ones (stride-16 AP), rhs = e5m2 exp pair-tiles.
  - finalize: acc,den scaled by 2^-12 into f16 (ACT), PE transposes back to
    [q, d], per-partition reciprocal of denT, out = (otW*wg*x + otB*bg)*rinv.
"""

import numpy as np

B = 4
C = 4096
D = 64
NCORES = 8
CQ = C // 2          # queries per core
KT = C // 128        # 32 k-tiles
NJ = KT // 2         # 16 k-tile pairs
QT = CQ // 128       # 16 q-tiles per core

# exp engine split per k-pair j (A=ACT, D=DVE), one pattern per chunk
import os as _os
EXP_PATTERNS = _os.environ.get(
    "EXP_PATTERNS", "ADADADADADADADAA,ADADADADADADADAA,"
    "ADADADADADADADAA,ADADADADADADAADA").split(",")
SQ_ENGINES = _os.environ.get("SQ_ENGINES", "PPPPPPPP")

LN2 = 0.6931471805599453
EXP_SCALE = 10.0
EXP_BIAS = -2.0
SCH_A = EXP_SCALE * 4.0 / LN2                    # 57.70780
SCH_B = 60.0 - 0.2316 + EXP_BIAS * 4.0 / LN2     # 48.2268
FIN_SCALE = 2.0 ** -12

_CACHE = {}


def _build(h):
    import concourse.bass as bass
    import concourse.bacc as bacc
    import concourse.mybir as mybir
    import concourse.tile as tile
    from concourse.masks import make_identity

    f32 = mybir.dt.float32
    f16 = mybir.dt.float16
    f8e4 = mybir.dt.float8e4
    f8e5 = mybir.dt.float8e5
    i32 = mybir.dt.int32
    u8 = mybir.dt.uint8
    u32 = mybir.dt.uint32
    AF = mybir.ActivationFunctionType
    ALU = mybir.AluOpType
    DR = mybir.MatmulPerfMode.DoubleRow

    qt0 = QT * h  # first k-tile of this core's query half

    nc = bacc.Bacc(None, target_bir_lowering=False)
    xk = nc.dram_tensor("xk", [C, D], f32, kind="ExternalInput")
    wcat = nc.dram_tensor("wcat", [C, 2 * D], f32, kind="ExternalInput")
    wg = nc.dram_tensor("wg", [CQ, D], f32, kind="ExternalInput")
    bg = nc.dram_tensor("bg", [CQ, D], f32, kind="ExternalInput")
    out = nc.dram_tensor("out", [CQ, D], f32, kind="ExternalOutput")

    # chunk order: this core's query-half k-tiles first
    corder = [2, 3, 0, 1] if h == 1 else [0, 1, 2, 3]
    torder = [t for c in corder for t in range(8 * c, 8 * c + 8)]
    jorder = [j for c in corder for j in range(4 * c, 4 * c + 4)]

    with tile.TileContext(nc) as tc:
        with (
            tc.tile_pool(name="singles", bufs=1) as singles,
            tc.tile_pool(name="sb", bufs=2) as sb,
            tc.tile_pool(name="exp", bufs=6) as expp,
            tc.tile_pool(name="fin", bufs=2) as fin,
            tc.tile_pool(name="stageA", bufs=2, space="PSUM") as stageA,
            tc.tile_pool(name="stageD", bufs=2, space="PSUM") as stageD,
            tc.tile_pool(name="acc", bufs=1, space="PSUM") as accp,
            tc.tile_pool(name="den", bufs=1, space="PSUM") as denp,
        ):
            # ---------------- tiles ----------------
            xk_r = xk.rearrange("(t p) d -> p t d", p=128)
            wc_r = wcat.rearrange("(t p) m -> p t m", p=128)
            out_r = out.rearrange("(t p) d -> p t d", p=128)
            kbig = singles.tile([128, KT, D], f32)
            wcs = singles.tile([128, KT, 2 * D], f32)
            wgs = singles.tile([128, QT, D], f32)
            bgs = singles.tile([128, QT, D], f32)
            identity = singles.tile([128, 128], f32)
            id4 = singles.tile([128, 128], f8e4)
            id16 = singles.tile([128, 128], f16)
            ones8 = singles.tile([128, 2, 16], f8e5)
            ebias = singles.tile([128, 1], f32)
            knT8 = singles.tile([64, 9 * 512], f8e4)
            knT3 = knT8.rearrange("p (a b) -> p a b", b=512)
            ksq = singles.tile([128, KT], f32)
            rsq = singles.tile([128, KT], f32)
            rtmp = singles.tile([128, KT], f32)
            rtmp2 = singles.tile([128, KT], f32)
            kn8 = singles.tile([128, KT, D], f8e4)
            wb8 = singles.tile([128, NJ, 2, 128], f8e4)
            wb8f = wb8.rearrange("p a t m -> p (a t m)")
            wcsf = wcs.rearrange("p t m -> p (t m)")
            wbq16 = singles.tile([128, QT, 2, D], f16)
            out_nat = singles.tile([128, QT, D], f32)

            # ---------------- DMA: first 8 k-tiles, then wcs c0 ----------
            def dma_kbig(t0, n):
                nc.sync.dma_start(out=kbig[:, t0 : t0 + n, :],
                                  in_=xk_r[:, t0 : t0 + n, :])

            dma_kbig(0, 4)
            dma_kbig(4, 4)
            dma_kbig(8, 8)
            dma_kbig(16, 8)
            dma_kbig(24, 8)
            for c in range(4):
                nc.sync.dma_start(out=wcs[:, 8 * c : 8 * (c + 1), :],
                                  in_=wc_r[:, 8 * c : 8 * (c + 1), :])
            nc.sync.dma_start(out=wgs, in_=wg.rearrange("(t p) d -> p t d", p=128))
            nc.sync.dma_start(out=bgs, in_=bg.rearrange("(t p) d -> p t d", p=128))

            # ---------------- constants ----------------
            make_identity(nc, identity)
            nc.gpsimd.tensor_copy(out=id4, in_=identity)
            nc.gpsimd.tensor_copy(out=id16, in_=identity)
            nc.vector.memset(ones8, 1.0)
            nc.vector.memset(ebias, EXP_BIAS)
            nc.vector.memset(knT8[:, 8 * 512 :].bitcast(u32), 0)

            # ---------------- per-4-tile prep: norms, kn8, transpose -------
            def prep_group(g, sq_engine):
                cs = slice(4 * g, 4 * (g + 1))
                ktmp = sb.tile([128, 4, D], f32, tag="ktmp", name=f"ktmp{g}")
                if sq_engine == "A":
                    nc.scalar.activation(out=ktmp, in_=kbig[:, cs, :],
                                         func=AF.Square)
                else:
                    nc.gpsimd.tensor_mul(ktmp, kbig[:, cs, :], kbig[:, cs, :])
                nc.vector.reduce_sum(out=ksq[:, cs], in_=ktmp,
                                     axis=mybir.AxisListType.X)
                ve = nc.vector
                ve.tensor_scalar(
                    out=rtmp[:, cs].bitcast(i32), in0=ksq[:, cs].bitcast(i32),
                    scalar1=1, scalar2=None, op0=ALU.arith_shift_right)
                ve.tensor_scalar(
                    out=rsq[:, cs].bitcast(i32), in0=rtmp[:, cs].bitcast(i32),
                    scalar1=-1, scalar2=0x5F3759DF, op0=ALU.mult, op1=ALU.add)
                ve.tensor_mul(rtmp[:, cs], rsq[:, cs], rsq[:, cs])
                ve.tensor_mul(rtmp2[:, cs], rtmp[:, cs], ksq[:, cs])
                ve.tensor_scalar(
                    out=rtmp2[:, cs], in0=rtmp2[:, cs],
                    scalar1=-0.5, scalar2=1.5, op0=ALU.mult, op1=ALU.add)
                ve.tensor_mul(rsq[:, cs], rsq[:, cs], rtmp2[:, cs])
                nc.vector.tensor_mul(
                    kn8[:, cs, :], kbig[:, cs, :],
                    rsq[:, cs].unsqueeze(2).broadcast_to([128, 4, D]))
                trp = stageA.tile([64, 4, 128, 2], f8e4, tag="stageA",
                                  name=f"trp{g}")
                for s in range(4):
                    nc.tensor.transpose(trp[:, s, :, 0], kn8[:, 4 * g + s, :], id4)
                dst = knT8[:, 4 * g * 128 : (4 * g + 4) * 128]
                if g % 4 != 3:
                    nc.scalar.copy(out=dst, in_=trp[:, :, :, 0])
                else:
                    nc.vector.tensor_copy(out=dst, in_=trp[:, :, :, 0])

            def wb_cast(c):
                cs = slice(512 * c, 512 * (c + 1))
                nc.gpsimd.tensor_copy(out=wb8f[:, cs], in_=wcsf[:, cs])

            # groups 0-1 + first weight cast up front
            prep_group(0, SQ_ENGINES[0])
            prep_group(1, SQ_ENGINES[1])
            wb_cast(0)
            for g in range(2, 8):
                prep_group(g, SQ_ENGINES[g])
                if g - 1 <= 7:
                    wb_cast(g - 1)
            wb_cast(7)
            nc.gpsimd.tensor_mul(wbq16[:, :, 0, :], wgs, kbig[:, qt0 : qt0 + QT, :])
            nc.gpsimd.tensor_copy(out=wbq16[:, :, 1, :], in_=bgs)

            # ---------------- finalize ----------------
            def fin_phase2(qc, half=None):
                acc16, den16 = fins[qc]
                key = f"{qc}" if half is None else f"{qc}_{half}"
                ot = stageA.tile([128, 2, 512], f32, tag="stageA",
                                 name=f"ot{key}")
                ot16f = ot.bitcast(f16)
                ot16 = ot16f.rearrange("p a b -> p (a b)")
                srange = (0, 1, 2, 3) if half is None else (2 * half, 2 * half + 1)
                for s in srange:
                    nc.tensor.transpose(
                        ot16[:, 128 * s : 128 * (s + 1)],
                        acc16[:, 128 * s : 128 * (s + 1)], id16)
                    nc.tensor.transpose(
                        ot16[:, 512 + 2 * s : 512 + 2 * s + 1],
                        den16[:, 128 * s : 128 * (s + 1)], id16[0:1, 0:1])
                nq = len(srange)
                s0_, q0_ = srange[0], 4 * qc + srange[0]
                rinvT = fin.tile([128, nq], f32, tag="rinvT", name=f"ri{key}")
                nc.vector.reciprocal(
                    out=rinvT,
                    in_=ot16[:, 512 + 2 * s0_ : 512 + 2 * s0_ + 2 * nq : 2])
                prod = fin.tile([128, nq, 128], f16, tag="prod", name=f"pr{key}")
                nc.vector.tensor_mul(
                    prod,
                    ot16[:, 128 * s0_ : 128 * (s0_ + nq)].rearrange(
                        "p (s m) -> p s m", s=nq),
                    wbq16[:, q0_ : q0_ + nq, :, :].rearrange(
                        "p a t m -> p a (t m)"))
                sumh = fin.tile([128, nq, D], f16, tag="sumh", name=f"sh{key}")
                nc.vector.tensor_add(sumh, prod[:, :, 0:D], prod[:, :, D:])
                nc.vector.tensor_mul(
                    out_nat[:, q0_ : q0_ + nq, :], sumh,
                    rinvT.unsqueeze(2).broadcast_to([128, nq, D]))
                nc.sync.dma_start(out=out_r[:, q0_ : q0_ + nq, :],
                                  in_=out_nat[:, q0_ : q0_ + nq, :])

            # ---------------- main loop ----------------
            fins = {}
            for qc in range(4):
                s0 = 4 * h + qc
                rhs = knT3[:, s0 : 9 : 8 - s0, :]
                acc_ps = accp.tile([128, 512], f32, tag="acc", name=f"acc{qc}")
                den_ps = denp.tile([1, 512], f32, tag="den", name=f"den{qc}")

                # schedule: group adjacent A-pairs in the pattern into quads
                pat = EXP_PATTERNS[qc]
                sched = []
                i = 0
                while i < NJ:
                    if False and pat[i] == "A" and i + 1 < NJ and pat[i + 1] == "A":
                        sched.append(("Q", (jorder[i], jorder[i + 1])))
                        i += 2
                    elif pat[i] == "A":
                        sched.append(("A", (jorder[i],)))
                        i += 1
                    else:
                        sched.append(("D", (jorder[i],)))
                        i += 1

                def lhsTs(j):
                    return (
                        knT8[:, 256 * j : 256 * j + 256].rearrange(
                            "p (t m) -> p t m", t=2),
                        knT8[:, 256 * j + 128 : 256 * j + 384].rearrange(
                            "p (t m) -> p t m", t=2))

                done = 0
                pending = []
                fin_emitted = qc == 0
                for kind, js in sched + [(None, ())]:
                    # drain stage-2/den for every e-pair already produced
                    for pj, e in pending:
                        done += 1
                        nc.tensor.matmul(
                            den_ps, lhsT=ones8[:, :, 0:1], rhs=e,
                            start=(done == 1), stop=(done == NJ),
                            perf_mode=DR, skip_group_check=True)
                        nc.tensor.matmul(
                            acc_ps, lhsT=wb8[:, pj], rhs=e,
                            start=(done == 1), stop=(done == NJ),
                            perf_mode=DR, skip_group_check=True)
                    pending = []
                    if not fin_emitted and done >= 7:
                        fin_phase2(qc - 1)
                        fin_emitted = True
                    if kind is None:
                        break
                    if kind == "Q":
                        j0, j1 = js
                        st = stageA.tile([128, 4, 512], f32, tag="stageA",
                                         name=f"st{qc}_{j0}q")
                        eq = expp.tile([128, 4, 512], f8e5, tag="exp",
                                       name=f"e{qc}_{j0}q")
                        for s, (jj, par) in enumerate(
                                ((j0, 0), (j0, 1), (j1, 0), (j1, 1))):
                            nc.tensor.matmul(
                                st[:, s, :], lhsT=lhsTs(jj)[par], rhs=rhs,
                                start=True, stop=True, perf_mode=DR)
                        nc.scalar.activation(out=eq, in_=st, func=AF.Exp,
                                             bias=ebias[:, 0:1],
                                             scale=EXP_SCALE)
                        pending = [(j0, eq[:, 0:2, :]), (j1, eq[:, 2:4, :])]
                    elif kind == "A":
                        (j,) = js
                        st = stageA.tile([128, 2, 512], f32, tag="stageA",
                                         name=f"st{qc}_{j}p")
                        e = expp.tile([128, 2, 512], f8e5, tag="exp",
                                      name=f"e{qc}_{j}")
                        l0, l1 = lhsTs(j)
                        nc.tensor.matmul(st[:, 0, :], lhsT=l0, rhs=rhs,
                                         start=True, stop=True, perf_mode=DR)
                        nc.tensor.matmul(st[:, 1, :], lhsT=l1, rhs=rhs,
                                         start=True, stop=True, perf_mode=DR)
                        nc.scalar.activation(out=e, in_=st, func=AF.Exp,
                                             bias=ebias[:, 0:1],
                                             scale=EXP_SCALE)
                        pending = [(j, e)]
                    else:
                        (j,) = js
                        e = expp.tile([128, 2, 512], f8e5, tag="exp",
                                      name=f"e{qc}_{j}")
                        l0, l1 = lhsTs(j)
                        for par, lh in ((0, l0), (1, l1)):
                            stx = stageD.tile([128, 512], f32, tag="stageD",
                                              name=f"st{qc}_{j}_{par}")
                            nc.tensor.matmul(stx, lhsT=lh, rhs=rhs,
                                             start=True, stop=True,
                                             perf_mode=DR)
                            nc.vector.tensor_scalar(
                                out=e[:, par, :].bitcast(u8), in0=stx,
                                scalar1=SCH_A, scalar2=SCH_B,
                                op0=ALU.mult, op1=ALU.add)
                        pending = [(j, e)]

                den16 = fin.tile([1, 512], f16, tag="den16", name=f"d16_{qc}")
                nc.scalar.activation(out=den16, in_=den_ps, func=AF.Copy,
                                     scale=FIN_SCALE)
                acc16 = fin.tile([128, 512], f16, tag="acc16", name=f"a16_{qc}")
                nc.scalar.activation(out=acc16, in_=acc_ps, func=AF.Copy,
                                     scale=FIN_SCALE)
                fins[qc] = (acc16, den16)
            fin_phase2(3, half=0)
            fin_phase2(3, half=1)

    nc.compile()
    return nc


def _get_nc(h):
    key = f"nc{h}"
    if key not in _CACHE:
        _CACHE[key] = _build(h)
    return _CACHE[key]


def _make_in_maps(input, weight, bias, weight_global, bias_global):
    input = np.ascontiguousarray(np.asarray(input, dtype=np.float32))
    ones = lambda: np.ones((C, D), np.float32)
    weight = ones() if weight is None else np.asarray(weight, np.float32)
    bias = np.zeros((C, D), np.float32) if bias is None else np.asarray(bias, np.float32)
    weight_global = ones() if weight_global is None else np.asarray(weight_global, np.float32)
    bias_global = ones() if bias_global is None else np.asarray(bias_global, np.float32)
    wcat = np.ascontiguousarray(np.concatenate([weight, bias], axis=1))
    in_maps = []
    for core in range(NCORES):
        b, h = divmod(core, 2)
        sl = slice(h * CQ, (h + 1) * CQ)
        in_maps.append({
            "xk": np.ascontiguousarray(input[b]),
            "wcat": wcat,
            "wg": np.ascontiguousarray(weight_global[sl]),
            "bg": np.ascontiguousarray(bias_global[sl]),
        })
    return in_maps


def _run(in_maps, **kw):
    from concourse.bass_utils import run_bass_kernel_spmd
    # h differs per core but all cores run one SPMD module; build for h via
    # per-core modules is not supported by run_bass_kernel_spmd -- so we run
    # the two halves as the same module parameterized by a dram flag? No:
    # instead we exploit that run_bass_kernel_spmd takes ONE nc. We fold h
    # into the inputs by rolling xk so every core behaves like h=0.
    nc = _get_nc(0)
    return run_bass_kernel_spmd(nc, in_maps, core_ids=list(range(NCORES)), **kw)


def kernel(input, weight=None, bias=None, weight_global=None, bias_global=None,
           **_ignored):
    in_maps = _make_in_maps(input, weight, bias, weight_global, bias_global)
    # roll keys so each core's query half sits at k-tiles 0..15
    for core in range(NCORES):
        b, h = divmod(core, 2)
        if h == 1:
            m = in_maps[core]
            m["xk"] = np.ascontiguousarray(np.roll(m["xk"], -CQ, axis=0))
            m["wcat"] = np.ascontiguousarray(np.roll(m["wcat"], -CQ, axis=0))
    res = _run(in_maps)
    out = np.empty((B, C, D), np.float32)
    for core in range(NCORES):
        b, h = divmod(core, 2)
        out[b, h * CQ : (h + 1) * CQ] = res.results[core]["out"]
    return out
